# revision 29
# baseline (speedup 1.0000x reference)
"""CalibreLoss TRN2 kernel v2: Act-queue-bound symmetric half-band SupCon.

Data-parallel over batch B across 8 cores. vs v1:
  * diag (d=0) and ring (d=64) tiles are folded into the 80-iteration band
    loop (one EXP activation per l, accum_out = row sums), cutting Act
    instruction count 120 -> 80 and removing separate s_d/s_r paths.
  * e_t / es are bf16 -> DVE es adds run in 2x mode (~half the time).
  * a 4-byte dummy collective issued first absorbs the CC-ring entry
    barrier; the feature AllGather is split in two halves so band l=16
    can start as soon as the first half lands.
  * the segment-sum AllReduce carries only [64,516] and runs during the
    band; row-sum partials (s_oth), colsums (sm), class dots (a), meta-CE
    logits (dn) etc. are shipped raw to the host, which does the final
    scatter/ln/softmax/NTXent assembly in numpy. No second AllReduce.
  * single act table set (exp/ln); sqrt/rsqrt eliminated everywhere.
"""

import sys

sys.path.insert(0, "/opt/trn_rl_repo")

import numpy as np

import concourse.bass as bass
import concourse.bacc as bacc
import concourse.mybir as mybir
import concourse.tile as tile
from concourse import bass_utils

F32 = mybir.dt.float32
BF16 = mybir.dt.bfloat16
I32 = mybir.dt.int32
AX = mybir.AxisListType
OP = mybir.AluOpType
AF = mybir.ActivationFunctionType

B = 8192
D = 128
K = 64
C = 10
T = 0.07
BT = 0.07
W_P = 0.5
W_N = 0.5
NC = 8
SH = B // NC          # 1024 rows of each input per core
NCH = SH // 128       # 8 chunks per input
NZ = 2 * NCH          # 16 z chunks (proj_a + proj_b)
LBAND = 80            # band iterations: l = 0..79 (rot blocks)
OUTW = 680
DUMMY_CC = False
PRS_GATHER = True
USE_TTR = False

_CACHE = {}


def _band_iter(nc, psb, ep, sp, featT, featR, es, s_oth, bexp, l):
    if l <= 15:
        wlo, whi = 0, l
        lhs = featT[:, l * 128:(l + 1) * 128]
    elif l <= 63:
        wlo, whi = 0, 15
        lhs = featR[:, (l - 16) * 128:(l - 15) * 128]
    else:
        wlo, whi = l - 64, 15
        lhs = featR[:, (l - 16) * 128:(l - 15) * 128]
    wid = (whi - wlo + 1) * 128
    psw = 1024 if l >= 72 else 2048
    ps = psb.tile([128, psw], F32, name="ps_f", tag="ps_f")
    c0 = wlo * 128
    cend = (whi + 1) * 128
    while c0 < cend:
        n = min(512, cend - c0)
        nc.tensor.matmul(ps[:, c0 - wlo * 128:c0 - wlo * 128 + n],
                         lhs, featT[:, c0:c0 + n])
        c0 += n
    e_t = ep.tile([128, 2048], BF16, name="e_t", tag="e_t")
    nc.scalar.activation(e_t[:, 0:wid], ps[:, 0:wid], AF.Exp,
                         scale=1.0 / T, bias=bexp[:],
                         accum_out=s_oth[:, l:l + 1])
    if 1 <= l <= 63:
        # skip trailing diag block for l<=15
        ew = (min(whi, l - 1) - wlo + 1) * 128
        nc.vector.tensor_add(es[:, wlo * 128:wlo * 128 + ew],
                             es[:, wlo * 128:wlo * 128 + ew],
                             e_t[:, 0:ew])
    elif l >= 64 and l < 79:
        # skip leading ring block
        ew = (whi - wlo) * 128
        nc.vector.tensor_add(es[:, (wlo + 1) * 128:(wlo + 1) * 128 + ew],
                             es[:, (wlo + 1) * 128:(wlo + 1) * 128 + ew],
                             e_t[:, 128:128 + ew])


def _build():
    nc = bacc.Bacc("TRN2", target_bir_lowering=False, debug=False, num_devices=NC)

    pa_d = nc.dram_tensor("pa", [SH, 128], F32, kind="ExternalInput")
    pb_d = nc.dram_tensor("pb", [SH, 128], F32, kind="ExternalInput")
    ea_d = nc.dram_tensor("ea", [SH, 128], F32, kind="ExternalInput")
    eb_d = nc.dram_tensor("eb", [SH, 128], F32, kind="ExternalInput")
    encT_d = nc.dram_tensor("encT", [128, 2048], F32, kind="ExternalInput")
    csc_d = nc.dram_tensor("centers_sc", [128, K], F32, kind="ExternalInput")
    csq_d = nc.dram_tensor("csq", [128, K], F32, kind="ExternalInput")
    labf_d = nc.dram_tensor("labels_f", [128, NCH], F32, kind="ExternalInput")
    i10r_d = nc.dram_tensor("iota10r", [128, C], F32, kind="ExternalInput")
    i64m_d = nc.dram_tensor("iota64m", [128, K], F32, kind="ExternalInput")
    eye_d = nc.dram_tensor("eye", [128, 128], F32, kind="ExternalInput")
    rotidx_d = nc.dram_tensor("rotidx", [128, 4], I32, kind="ExternalInput")

    out_d = nc.dram_tensor("out", [128, OUTW], F32, kind="ExternalOutput")
    out2_d = nc.dram_tensor("out2", [64, 515], F32, kind="ExternalOutput")

    with tile.TileContext(nc) as tc:
        with (
            tc.tile_pool(name="persist", bufs=1) as pp,
            tc.tile_pool(name="scratch", bufs=4) as sp,
            tc.tile_pool(name="etp", bufs=4) as ep,
            tc.tile_pool(name="dram", bufs=1, space="DRAM") as dp,
        ):
            featT = pp.tile([128, 2048], BF16, name="featT")
            featR = pp.tile([128, 8192], BF16, name="featR")
            z_rows = pp.tile([128, NZ * 129], F32, name="z_rows")
            rows_a = pp.tile([128, NCH * 257], F32, name="rows_a")
            rows_b = pp.tile([128, NCH * 129], F32, name="rows_b")
            eb_sb = pp.tile([128, SH], F32, name="eb_sb")
            encT_sb = pp.tile([128, 2048], F32, name="encT_sb")
            nsq_all = pp.tile([128, NZ], F32, name="nsq_all")
            lnq = pp.tile([128, NZ], F32, name="lnq")
            rinv_all = pp.tile([128, NZ], F32, name="rinv_all")
            sd_all = pp.tile([128, NZ], F32, name="sd_all")
            xsq_b = pp.tile([128, NCH], F32, name="xsq_b")
            idxm_all = pp.tile([128, NZ], F32, name="idxm_all")
            cl_oh = pp.tile([128, NZ * K], F32, name="cl_oh")
            s_oth = pp.tile([128, LBAND], F32, name="s_oth")
            sm_sb = pp.tile([128, 16], F32, name="sm_sb")
            es = pp.tile([128, 2048], BF16, name="es")
            dn_sb = pp.tile([128, NCH * K], F32, name="dn_sb")
            seg_a_sb = pp.tile([64, 257], F32, name="seg_a_sb")
            sup = pp.tile([64, 128], F32, name="sup")
            supT2 = pp.tile([128, 64], F32, name="supT2")
            csc = pp.tile([128, K], F32, name="csc_sb")
            csq = pp.tile([128, K], F32, name="csq_sb")
            labf = pp.tile([128, NCH], F32, name="labf_sb")
            i10r = pp.tile([128, C], F32, name="i10r_sb")
            i64m = pp.tile([128, K], F32, name="i64m_sb")
            eye = pp.tile([128, 128], F32, name="eye_sb")
            ones_bf = pp.tile([128, 1], BF16, name="ones_bf")
            bexp = pp.tile([128, 1], F32, name="bexp")
            rotidx = pp.tile([128, 4], I32, name="rotidx_sb")
            seg_b_sb = pp.tile([64, 129], F32, name="seg_b_sb")
            if DUMMY_CC:
                dm_sb = pp.tile([64, 129], F32, name="dm_sb")
                d_in = dp.tile([64, 129], F32, name="d_in")
                d_out = dp.tile([64, 129], F32, name="d_out", addr_space="Shared")
            ag1_in = dp.tile([128, 1024], BF16, name="ag1_in")
            ag1_out = dp.tile([128 * NC, 1024], BF16, name="ag1_out", addr_space="Shared")
            ag2_in = dp.tile([128, 1024], BF16, name="ag2_in")
            ag2_out = dp.tile([128 * NC, 1024], BF16, name="ag2_out", addr_space="Shared")
            ar_in = dp.tile([64, 516], F32, name="ar_in")
            ar_out = dp.tile([64, 516], F32, name="ar_out", addr_space="Shared")

            # ---- dummy collective first: absorbs CC entry barrier ----
            if DUMMY_CC:
                nc.vector.memset(dm_sb[:], 0.0)
                nc.sync.dma_start(d_in[:], dm_sb[:])
                nc.gpsimd.collective_compute(
                    "AllReduce", OP.add, replica_groups=[list(range(NC))],
                    ins=[d_in[:]], outs=[d_out[:]],
                )

            # ---- input loads (single strided DMAs; pa/pb first for AG path) ----
            ra3 = rows_a[:].rearrange("p (c w) -> p c w", c=NCH)
            rb3 = rows_b[:].rearrange("p (c w) -> p c w", c=NCH)
            eb3 = eb_sb[:].rearrange("p (c w) -> p c w", c=NCH)
            nc.sync.dma_start(ra3[:, :, 0:128],
                              pa_d[:].rearrange("(c p) d -> p c d", c=NCH))
            nc.sync.dma_start(rb3[:, :, 0:128],
                              pb_d[:].rearrange("(c p) d -> p c d", c=NCH))
            nc.sync.dma_start(ra3[:, :, 128:256],
                              ea_d[:].rearrange("(c p) d -> p c d", c=NCH))
            nc.sync.dma_start(eb3[:, :, :],
                              eb_d[:].rearrange("(c p) d -> p c d", c=NCH))
            for ch in range(NCH):
                nc.vector.memset(rows_a[:, ch * 257 + 256:ch * 257 + 257], 1.0)
                nc.vector.memset(rows_b[:, ch * 129 + 128:ch * 129 + 129], 1.0)
            nc.sync.dma_start(encT_sb[:], encT_d[:])
            nc.sync.dma_start(csc[:], csc_d[:])
            nc.sync.dma_start(csq[:], csq_d[:])
            nc.sync.dma_start(labf[:], labf_d[:])
            nc.sync.dma_start(i10r[:], i10r_d[:])
            nc.sync.dma_start(i64m[:], i64m_d[:])
            nc.sync.dma_start(eye[:], eye_d[:])
            nc.sync.dma_start(rotidx[:], rotidx_d[:])
            nc.vector.memset(bexp[:], -1.0 / T)
            nc.vector.memset(ones_bf[:], 1.0)
            nc.vector.memset(es[:], 0.0)
            for ch in range(NZ):
                nc.vector.memset(z_rows[:, ch * 129 + 128:ch * 129 + 129], 1.0)

            # ---- phase 1: normalize + transpose + split AllGather ----
            with tc.tile_pool(name="ps_pre", bufs=2, space="PSUM") as psp:
                for h in range(2):
                    for ch in range(h * 8, h * 8 + 8):
                        if ch < NCH:
                            src = rows_a[:, ch * 257:ch * 257 + 128]
                        else:
                            src = rows_b[:, (ch - NCH) * 129:(ch - NCH) * 129 + 128]
                        sq_s = sp.tile([128, 128], F32, name="sq_s", tag="sq_s")
                        nc.scalar.activation(sq_s[:], src, AF.Square,
                                             accum_out=nsq_all[:, ch:ch + 1])
                    h8 = h * 8
                    nc.scalar.activation(lnq[:, h8:h8 + 8], nsq_all[:, h8:h8 + 8], AF.Ln)
                    nc.scalar.activation(rinv_all[:, h8:h8 + 8], lnq[:, h8:h8 + 8],
                                         AF.Exp, scale=-0.5)
                    for ch in range(h * 8, h * 8 + 8):
                        if ch < NCH:
                            src = rows_a[:, ch * 257:ch * 257 + 128]
                        else:
                            src = rows_b[:, (ch - NCH) * 129:(ch - NCH) * 129 + 128]
                        zsl = z_rows[:, ch * 129:ch * 129 + 128]
                        nc.vector.tensor_scalar_mul(zsl, src, rinv_all[:, ch:ch + 1])
                        tr_ps = psp.tile([128, 128], F32, name="tr_ps", tag="tr_ps")
                        nc.tensor.transpose(tr_ps[:], zsl, eye[:])
                        nc.vector.tensor_copy(featT[:, ch * 128:(ch + 1) * 128], tr_ps[:])
                        zbf = sp.tile([128, 128], BF16, name="zbf", tag="zbf")
                        nc.vector.tensor_copy(zbf[:], zsl)
                        zb2 = sp.tile([128, 128], F32, name="zb2", tag="zb2")
                        nc.vector.tensor_mul(zb2[:], zbf[:], zbf[:])
                        nc.vector.tensor_reduce(sd_all[:, ch:ch + 1], zb2[:],
                                                AX.X, OP.add)
                    if h == 0:
                        nc.sync.dma_start(ag1_in[:], featT[:, 0:1024])
                        nc.gpsimd.collective_compute(
                            "AllGather", OP.bypass, replica_groups=[list(range(NC))],
                            ins=[ag1_in[:]], outs=[ag1_out[:]],
                        )
                    else:
                        nc.sync.dma_start(ag2_in[:], featT[:, 1024:2048])
                        nc.gpsimd.collective_compute(
                            "AllGather", OP.bypass, replica_groups=[list(range(NC))],
                            ins=[ag2_in[:]], outs=[ag2_out[:]],
                        )

            # ---- band A: l = 0..15 (featR-free intra tiles) ----
            with tc.tile_pool(name="ps_bandA", bufs=2, space="PSUM") as psba:
                for l in range(16):
                    _band_iter(nc, psba, ep, sp, featT, featR, es, s_oth, bexp, l)
            for ch in range(NCH):
                sq2 = sp.tile([128, 128], F32, name="sq2", tag="sq_s")
                nc.scalar.activation(sq2[:], eb_sb[:, ch * 128:(ch + 1) * 128],
                                     AF.Square, accum_out=xsq_b[:, ch:ch + 1])

            # ---- phase 2: assignment + one-hots + seg sums + seg AllReduce ----
            with tc.tile_pool(name="ps_ph2", bufs=2, space="PSUM") as psp2:
                for ch in range(NZ):
                    sc_ps = psp2.tile([128, K], F32, name="sc_ps", tag="sc_ps")
                    nc.tensor.matmul(sc_ps[:], encT_sb[:, ch * 128:(ch + 1) * 128], csc[:])
                    s_sb = sp.tile([128, K], F32, name="s_sb", tag="s_sb")
                    nc.vector.scalar_tensor_tensor(s_sb[:], sc_ps[:], 1.0, csq[:],
                                                   op0=OP.mult, op1=OP.add)
                    mn = sp.tile([128, 1], F32, name="mn", tag="mn")
                    nc.vector.tensor_reduce(mn[:], s_sb[:], AX.X, OP.min)
                    tmp = sp.tile([128, K], F32, name="tmp", tag="tmp")
                    nc.vector.scalar_tensor_tensor(tmp[:], s_sb[:], mn[:], i64m[:],
                                                   op0=OP.is_equal, op1=OP.mult)
                    nc.vector.tensor_reduce(idxm_all[:, ch:ch + 1], tmp[:], AX.X, OP.min)
                    nc.vector.tensor_scalar(cl_oh[:, ch * K:(ch + 1) * K], i64m[:],
                                            idxm_all[:, ch:ch + 1], None, op0=OP.is_equal)
                with tc.tile_pool(name="ps_seg", bufs=1, space="PSUM") as pseg:
                    seg_a = pseg.tile([64, 257], F32, name="seg_a")
                    seg_b = pseg.tile([64, 129], F32, name="seg_b")
                    p_ps = pseg.tile([C, 129], F32, name="p_ps")
                    for ch in range(NCH):
                        nc.tensor.matmul(seg_a[:], cl_oh[:, ch * K:(ch + 1) * K],
                                         rows_a[:, ch * 257:(ch + 1) * 257],
                                         start=(ch == 0), stop=(ch == NCH - 1))
                    for ch in range(NCH):
                        oh = cl_oh[:, (NCH + ch) * K:(NCH + ch + 1) * K]
                        nc.tensor.matmul(seg_b[:], oh, rows_b[:, ch * 129:(ch + 1) * 129],
                                         start=(ch == 0), stop=(ch == NCH - 1))
                    for ch in range(NZ):
                        loh = sp.tile([128, C], F32, name="loh", tag="loh")
                        nc.vector.tensor_scalar(loh[:], i10r[:],
                                                labf[:, (ch % NCH):(ch % NCH) + 1],
                                                None, op0=OP.is_equal)
                        nc.tensor.matmul(p_ps[:], loh[:], z_rows[:, ch * 129:(ch + 1) * 129],
                                         start=(ch == 0), stop=(ch == NZ - 1))
                    st_a = sp.tile([64, 257], F32, name="st_a", tag="st_a")
                    nc.vector.tensor_copy(st_a[:], seg_a[:])
                    st_b = sp.tile([64, 129], F32, name="st_b", tag="st_b")
                    nc.vector.tensor_copy(st_b[:], seg_b[:])
                    st_p = sp.tile([C, 129], F32, name="st_p", tag="st_p")
                    nc.vector.tensor_copy(st_p[:], p_ps[:])
                    nc.sync.dma_start(ar_in[0:C, 0:129], st_p[:])
                    nc.sync.dma_start(ar_in[0:64, 129:386], st_a[:])
                    nc.sync.dma_start(ar_in[0:64, 386:515], st_b[:])
            # ---- featR: rotated gather of cores r+1..r+4 from the two AGs ----
            for q in range(1, 5):
                base = (q - 1) * 2048
                nc.gpsimd.indirect_dma_start(
                    out=featR[:, base:base + 1024], out_offset=None,
                    in_=ag1_out[:],
                    in_offset=bass.IndirectOffsetOnAxis(ap=rotidx[:, q - 1:q], axis=0),
                )
                nc.gpsimd.indirect_dma_start(
                    out=featR[:, base + 1024:base + 2048], out_offset=None,
                    in_=ag2_out[:],
                    in_offset=bass.IndirectOffsetOnAxis(ap=rotidx[:, q - 1:q], axis=0),
                )
            nc.gpsimd.collective_compute(
                "AllReduce", OP.add, replica_groups=[list(range(NC))],
                ins=[ar_in[:]], outs=[ar_out[:]],
            )

            # ---- band B: l = 16..71 full-width ----
            with tc.tile_pool(name="ps_band", bufs=2, space="PSUM") as psb:
                for l in range(16, 72):
                    _band_iter(nc, psb, ep, sp, featT, featR, es, s_oth, bexp, l)

            # ---- tail part A (overlaps final narrow band iters) ----
            with tc.tile_pool(name="ps_tail", bufs=1, space="PSUM") as pst:
                for t in range(8):
                    smt = pst.tile([128, 1], F32, name="smt", tag="smt")
                    nc.tensor.matmul(smt[:], es[:, t * 128:(t + 1) * 128], ones_bf[:])
                    nc.vector.tensor_copy(sm_sb[:, t:t + 1], smt[:])
                nc.sync.dma_start(seg_a_sb[:], ar_out[0:64, 129:386])
                ca = sp.tile([64, 1], F32, name="ca", tag="ca")
                nc.vector.tensor_scalar_max(ca[:], seg_a_sb[:, 256:257], 1.0)
                rac = sp.tile([64, 1], F32, name="rac", tag="rac")
                nc.vector.reciprocal(rac[:], ca[:])
                nc.vector.tensor_scalar_mul(sup[:], seg_a_sb[:, 128:256], rac[:])
                sup_tp = pst.tile([128, 64], F32, name="sup_tp", tag="p1")
                nc.tensor.transpose(sup_tp[:], sup[:], eye[0:64, 0:64])
                nc.vector.tensor_scalar_mul(supT2[:], sup_tp[:], -2.0)
                for ch in range(NCH):
                    dn_ps = pst.tile([128, K], F32, name="dn_ps", tag="p2")
                    nc.tensor.matmul(dn_ps[:],
                                     encT_sb[:, (NCH + ch) * 128:(NCH + ch + 1) * 128],
                                     supT2[:])
                    nc.vector.tensor_copy(dn_sb[:, ch * K:(ch + 1) * K], dn_ps[:])
                nc.sync.dma_start(out_d[:, 0:512], dn_sb[:])
                nc.sync.dma_start(out_d[:, 640:656], sd_all[:])
                nc.sync.dma_start(out_d[:, 656:664], xsq_b[:])
                nc.sync.dma_start(out_d[:, 664:680], idxm_all[:])
                nc.sync.dma_start(seg_b_sb[:], ar_out[0:64, 386:515])
                p_stage = sp.tile([C, 129], F32, name="p_stage", tag="p_stage")
                nc.sync.dma_start(p_stage[:], ar_out[0:C, 0:129])
                nc.sync.dma_start(out2_d[:, 0:257], seg_a_sb[:])
                nc.sync.dma_start(out2_d[:, 257:386], seg_b_sb[:])
                nc.sync.dma_start(out2_d[0:C, 386:515], p_stage[:])

                # ---- band C: l = 72..79 narrow (1024-wide psum) ----
                with tc.tile_pool(name="ps_band2", bufs=2, space="PSUM") as psb2:
                    for l in range(72, LBAND):
                        _band_iter(nc, psb2, ep, sp, featT, featR, es, s_oth, bexp, l)

                for t in range(8, 16):
                    smt = pst.tile([128, 1], F32, name="smt", tag="smt")
                    nc.tensor.matmul(smt[:], es[:, t * 128:(t + 1) * 128], ones_bf[:])
                    nc.vector.tensor_copy(sm_sb[:, t:t + 1], smt[:])
                nc.sync.dma_start(out_d[:, 512:592], s_oth[:])
                nc.sync.dma_start(out_d[:, 592:608], sm_sb[:])

    nc.compile()
    return nc


def _prep_inputs(encodings_a, encodings_b, projections_a, projections_b,
                 cluster_centers, labels):
    ea = np.ascontiguousarray(encodings_a, dtype=np.float32)
    eb = np.ascontiguousarray(encodings_b, dtype=np.float32)
    pa = np.ascontiguousarray(projections_a, dtype=np.float32)
    pb = np.ascontiguousarray(projections_b, dtype=np.float32)
    cc = np.ascontiguousarray(cluster_centers, dtype=np.float32)
    lab = np.asarray(labels).astype(np.float32)

    csc = np.ascontiguousarray((-2.0 * cc).T)
    csq = np.tile(np.sum(cc * cc, axis=1)[None, :], (128, 1)).astype(np.float32)
    i10r = np.tile(np.arange(C, dtype=np.float32)[None, :], (128, 1))
    i64m = np.tile((np.arange(K, dtype=np.float32) - K)[None, :], (128, 1))
    eye = np.eye(128, dtype=np.float32)

    _CACHE["aux"] = {"pa": pa, "pb": pb, "lab": lab.astype(np.int64)}
    in_maps = []
    for r in range(NC):
        s = slice(r * SH, (r + 1) * SH)
        labs = lab[s]
        ri = np.stack([((r + 1 + j) % NC) * 128 + np.arange(128) for j in range(4)],
                      axis=1).astype(np.int32)
        labc = labs.reshape(NCH, 128).T.astype(np.float32)   # [128, 8]
        in_maps.append({
            "pa": pa[s], "pb": pb[s], "ea": ea[s], "eb": eb[s],
            "encT": np.ascontiguousarray(np.concatenate([ea[s], eb[s]], 0).T),
            "centers_sc": csc, "csq": csq,
            "labels_f": np.ascontiguousarray(labc),
            "iota10r": i10r, "iota64m": i64m, "eye": eye,
            "rotidx": ri,
        })
    return in_maps


def _combine(results):
    outs = [np.asarray(res["out"], dtype=np.float64) for res in results]
    seg = np.asarray(results[0]["out2"], dtype=np.float64)

    # ---- l_main: scatter row-sum partials, assemble log-denominators ----
    aux = _CACHE["aux"]
    P = seg[0:C, 386:515]                    # class sums [10, 129]
    S_glob = np.zeros((128, 128))  # [block, row-in-block]
    for r in range(NC):
        s_oth = outs[r][:, 512:592]          # [128(p), 80(l)]
        g = (16 * r + np.arange(LBAND)) % 128
        np.add.at(S_glob, g, s_oth.T)
    l_main_sum = 0.0
    for r in range(NC):
        s = slice(r * SH, (r + 1) * SH)
        za = aux["pa"][s] / np.linalg.norm(aux["pa"][s], axis=1, keepdims=True)
        zb = aux["pb"][s] / np.linalg.norm(aux["pb"][s], axis=1, keepdims=True)
        zr = np.concatenate([za, zb], 0).astype(np.float64)       # [2048, 128]
        labs2 = np.concatenate([aux["lab"][s], aux["lab"][s]])
        a = np.sum(zr * P[labs2, 0:128], axis=1).reshape(NZ, 128).T
        n2 = P[labs2, 128].reshape(NZ, 128).T
        sm = outs[r][:, 592:608]             # [128, 16] col-side sums (my rows)
        sd = outs[r][:, 640:656]
        S = S_glob[16 * r:16 * r + 16].T + sm - np.exp((sd - 1.0) / T)
        m1 = (a - n2) / T / (n2 - 1.0)
        l_main_sum += np.sum(m1 - np.log(S))
    l_main = -(T / BT) * l_main_sum / (2 * B)

    # ---- l_p: prototype NTXent on host (64-dim, trivial) ----
    ca = np.maximum(seg[:, 256], 1.0)
    cb = np.maximum(seg[:, 385], 1.0)
    proto_a = seg[:, 0:128] / ca[:, None]
    proto_b = seg[:, 257:385] / cb[:, None]
    za = proto_a / np.linalg.norm(proto_a, axis=1, keepdims=True)
    zb = proto_b / np.linalg.norm(proto_b, axis=1, keepdims=True)
    z = np.concatenate([za, zb], 0)
    n = 2 * K
    sim = (z @ z.T) / T
    np.fill_diagonal(sim, -np.inf)
    pos = (np.arange(n) + K) % n
    mx = np.max(sim, axis=1, keepdims=True)
    logp = sim - mx - np.log(np.sum(np.exp(sim - mx), axis=1, keepdims=True))
    l_p = -np.mean(logp[np.arange(n), pos])

    # ---- l_n: meta CE from shipped -2*e.sup logits ----
    sup_v = seg[:, 128:256] / ca[:, None]
    ssq = np.sum(sup_v * sup_v, axis=1)
    l_n_sum = 0.0
    for r in range(NC):
        dn = outs[r][:, 0:512].reshape(128, NCH, K)
        xsq = outs[r][:, 656:664]
        lb = (outs[r][:, 664:680] + K)[:, NCH:].astype(np.int64)  # [128, 8]
        d2 = dn + xsq[:, :, None] + ssq[None, None, :]
        dd = np.sqrt(np.maximum(d2, 0.0))
        mxd = np.min(dd, axis=2, keepdims=True)
        ls = -(dd - mxd) - np.log(np.sum(np.exp(-(dd - mxd)), axis=2, keepdims=True))
        p_idx, c_idx = np.meshgrid(np.arange(128), np.arange(NCH), indexing="ij")
        l_n_sum += np.sum(ls[p_idx, c_idx, lb])
    l_n = -l_n_sum / B

    return np.float32(l_main + W_P * l_p + W_N * l_n)


def kernel(encodings_a, encodings_b, projections_a, projections_b,
           cluster_centers, labels):
    if "nc" not in _CACHE:
        _CACHE["nc"] = _build()
    nc = _CACHE["nc"]
    in_maps = _prep_inputs(encodings_a, encodings_b, projections_a,
                           projections_b, cluster_centers, labels)
    res = bass_utils.run_bass_kernel_spmd(nc, in_maps, core_ids=list(range(NC)))
    return _combine(res.results)


# revision 31
# speedup vs baseline: 1.0427x; 1.0427x over previous
"""CalibreLoss TRN2 kernel v2: Act-queue-bound symmetric half-band SupCon.

Data-parallel over batch B across 8 cores. vs v1:
  * diag (d=0) and ring (d=64) tiles are folded into the 80-iteration band
    loop (one EXP activation per l, accum_out = row sums), cutting Act
    instruction count 120 -> 80 and removing separate s_d/s_r paths.
  * e_t / es are bf16 -> DVE es adds run in 2x mode (~half the time).
  * a 4-byte dummy collective issued first absorbs the CC-ring entry
    barrier; the feature AllGather is split in two halves so band l=16
    can start as soon as the first half lands.
  * the segment-sum AllReduce carries only [64,516] and runs during the
    band; row-sum partials (s_oth), colsums (sm), class dots (a), meta-CE
    logits (dn) etc. are shipped raw to the host, which does the final
    scatter/ln/softmax/NTXent assembly in numpy. No second AllReduce.
  * single act table set (exp/ln); sqrt/rsqrt eliminated everywhere.
"""

import sys

sys.path.insert(0, "/opt/trn_rl_repo")

import numpy as np

import concourse.bass as bass
import concourse.bacc as bacc
import concourse.mybir as mybir
import concourse.tile as tile
from concourse import bass_utils

F32 = mybir.dt.float32
BF16 = mybir.dt.bfloat16
I32 = mybir.dt.int32
AX = mybir.AxisListType
OP = mybir.AluOpType
AF = mybir.ActivationFunctionType

B = 8192
D = 128
K = 64
C = 10
T = 0.07
BT = 0.07
W_P = 0.5
W_N = 0.5
NC = 8
SH = B // NC          # 1024 rows of each input per core
NCH = SH // 128       # 8 chunks per input
NZ = 2 * NCH          # 16 z chunks (proj_a + proj_b)
LBAND = 80            # band iterations: l = 0..79 (rot blocks)
OUTW = 680
DUMMY_CC = False
PRS_GATHER = True
USE_TTR = False

_CACHE = {}


def _band_iter(nc, psb, ep, sp, featT, featR, es, s_oth, bexp, l):
    if l <= 15:
        wlo, whi = 0, l
        lhs = featT[:, l * 128:(l + 1) * 128]
    elif l <= 63:
        wlo, whi = 0, 15
        lhs = featR[:, (l - 16) * 128:(l - 15) * 128]
    else:
        wlo, whi = l - 64, 15
        lhs = featR[:, (l - 16) * 128:(l - 15) * 128]
    wid = (whi - wlo + 1) * 128
    psw = 1024 if l >= 72 else 2048
    ps = psb.tile([128, psw], F32, name="ps_f", tag="ps_f")
    c0 = wlo * 128
    cend = (whi + 1) * 128
    while c0 < cend:
        n = min(512, cend - c0)
        nc.tensor.matmul(ps[:, c0 - wlo * 128:c0 - wlo * 128 + n],
                         lhs, featT[:, c0:c0 + n])
        c0 += n
    e_t = ep.tile([128, 2048], BF16, name="e_t", tag="e_t")
    nc.scalar.activation(e_t[:, 0:wid], ps[:, 0:wid], AF.Exp,
                         scale=1.0 / T, bias=bexp[:],
                         accum_out=s_oth[:, l:l + 1])
    if 1 <= l <= 63:
        # skip trailing diag block for l<=15
        ew = (min(whi, l - 1) - wlo + 1) * 128
        nc.vector.tensor_add(es[:, wlo * 128:wlo * 128 + ew],
                             es[:, wlo * 128:wlo * 128 + ew],
                             e_t[:, 0:ew])
    elif l >= 64 and l < 79:
        # skip leading ring block
        ew = (whi - wlo) * 128
        nc.vector.tensor_add(es[:, (wlo + 1) * 128:(wlo + 1) * 128 + ew],
                             es[:, (wlo + 1) * 128:(wlo + 1) * 128 + ew],
                             e_t[:, 128:128 + ew])


def _build():
    nc = bacc.Bacc("TRN2", target_bir_lowering=False, debug=False, num_devices=NC)

    pa_d = nc.dram_tensor("pa", [SH, 128], F32, kind="ExternalInput")
    pb_d = nc.dram_tensor("pb", [SH, 128], F32, kind="ExternalInput")
    ea_d = nc.dram_tensor("ea", [SH, 128], F32, kind="ExternalInput")
    eb_d = nc.dram_tensor("eb", [SH, 128], F32, kind="ExternalInput")
    encT_d = nc.dram_tensor("encT", [128, 2048], F32, kind="ExternalInput")
    csc_d = nc.dram_tensor("centers_sc", [128, K], F32, kind="ExternalInput")
    csq_d = nc.dram_tensor("csq", [128, K], F32, kind="ExternalInput")
    labf_d = nc.dram_tensor("labels_f", [128, NCH], F32, kind="ExternalInput")
    i10r_d = nc.dram_tensor("iota10r", [128, C], F32, kind="ExternalInput")
    i64m_d = nc.dram_tensor("iota64m", [128, K], F32, kind="ExternalInput")
    eye_d = nc.dram_tensor("eye", [128, 128], F32, kind="ExternalInput")
    rotidx_d = nc.dram_tensor("rotidx", [128, 4], I32, kind="ExternalInput")

    out_d = nc.dram_tensor("out", [128, OUTW], F32, kind="ExternalOutput")
    out2_d = nc.dram_tensor("out2", [64, 515], F32, kind="ExternalOutput")

    with tile.TileContext(nc) as tc:
        with (
            tc.tile_pool(name="persist", bufs=1) as pp,
            tc.tile_pool(name="scratch", bufs=4) as sp,
            tc.tile_pool(name="etp", bufs=4) as ep,
            tc.tile_pool(name="dram", bufs=1, space="DRAM") as dp,
        ):
            featT = pp.tile([128, 2048], BF16, name="featT")
            featR = pp.tile([128, 8192], BF16, name="featR")
            z_rows = pp.tile([128, NZ * 129], F32, name="z_rows")
            rows_a = pp.tile([128, NCH * 257], F32, name="rows_a")
            rows_b = pp.tile([128, NCH * 129], F32, name="rows_b")
            eb_sb = pp.tile([128, SH], F32, name="eb_sb")
            encT_sb = pp.tile([128, 2048], F32, name="encT_sb")
            nsq_all = pp.tile([128, NZ], F32, name="nsq_all")
            lnq = pp.tile([128, NZ], F32, name="lnq")
            rinv_all = pp.tile([128, NZ], F32, name="rinv_all")
            sd_all = pp.tile([128, NZ], F32, name="sd_all")
            xsq_b = pp.tile([128, NCH], F32, name="xsq_b")
            idxm_all = pp.tile([128, NZ], F32, name="idxm_all")
            cl_oh = pp.tile([128, NZ * K], F32, name="cl_oh")
            s_oth = pp.tile([128, LBAND], F32, name="s_oth")
            sm_sb = pp.tile([128, 16], F32, name="sm_sb")
            es = pp.tile([128, 2048], BF16, name="es")
            dn_sb = pp.tile([128, NCH * K], F32, name="dn_sb")
            seg_a_sb = pp.tile([64, 257], F32, name="seg_a_sb")
            sup = pp.tile([64, 128], F32, name="sup")
            supT2 = pp.tile([128, 64], F32, name="supT2")
            csc = pp.tile([128, K], F32, name="csc_sb")
            csq = pp.tile([128, K], F32, name="csq_sb")
            labf = pp.tile([128, NCH], F32, name="labf_sb")
            i10r = pp.tile([128, C], F32, name="i10r_sb")
            i64m = pp.tile([128, K], F32, name="i64m_sb")
            eye = pp.tile([128, 128], F32, name="eye_sb")
            ones_bf = pp.tile([128, 1], BF16, name="ones_bf")
            bexp = pp.tile([128, 1], F32, name="bexp")
            rotidx = pp.tile([128, 4], I32, name="rotidx_sb")
            seg_b_sb = pp.tile([64, 129], F32, name="seg_b_sb")
            if DUMMY_CC:
                dm_sb = pp.tile([64, 129], F32, name="dm_sb")
                d_in = dp.tile([64, 129], F32, name="d_in")
                d_out = dp.tile([64, 129], F32, name="d_out", addr_space="Shared")
            ag1_in = dp.tile([128, 1024], BF16, name="ag1_in")
            ag1_out = dp.tile([128 * NC, 1024], BF16, name="ag1_out", addr_space="Shared")
            ag2_in = dp.tile([128, 1024], BF16, name="ag2_in")
            ag2_out = dp.tile([128 * NC, 1024], BF16, name="ag2_out", addr_space="Shared")
            ar_in = dp.tile([64, 516], F32, name="ar_in")
            ar_out = dp.tile([64, 516], F32, name="ar_out", addr_space="Shared")

            # ---- dummy collective first: absorbs CC entry barrier ----
            if DUMMY_CC:
                nc.vector.memset(dm_sb[:], 0.0)
                nc.sync.dma_start(d_in[:], dm_sb[:])
                nc.gpsimd.collective_compute(
                    "AllReduce", OP.add, replica_groups=[list(range(NC))],
                    ins=[d_in[:]], outs=[d_out[:]],
                )

            # ---- input loads (pa/pb per-chunk so phase1 pipelines with DMA) ----
            ra3 = rows_a[:].rearrange("p (c w) -> p c w", c=NCH)
            eb3 = eb_sb[:].rearrange("p (c w) -> p c w", c=NCH)
            for ch in range(NCH):
                nc.sync.dma_start(rows_a[:, ch * 257:ch * 257 + 128],
                                  pa_d[ch * 128:(ch + 1) * 128, :])
            for ch in range(NCH):
                nc.sync.dma_start(rows_b[:, ch * 129:ch * 129 + 128],
                                  pb_d[ch * 128:(ch + 1) * 128, :])
            nc.sync.dma_start(ra3[:, :, 128:256],
                              ea_d[:].rearrange("(c p) d -> p c d", c=NCH))
            nc.sync.dma_start(eb3[:, :, :],
                              eb_d[:].rearrange("(c p) d -> p c d", c=NCH))
            for ch in range(NCH):
                nc.vector.memset(rows_a[:, ch * 257 + 256:ch * 257 + 257], 1.0)
                nc.vector.memset(rows_b[:, ch * 129 + 128:ch * 129 + 129], 1.0)
            nc.sync.dma_start(encT_sb[:], encT_d[:])
            nc.sync.dma_start(csc[:], csc_d[:])
            nc.sync.dma_start(csq[:], csq_d[:])
            nc.sync.dma_start(labf[:], labf_d[:])
            nc.sync.dma_start(i10r[:], i10r_d[:])
            nc.sync.dma_start(i64m[:], i64m_d[:])
            nc.sync.dma_start(eye[:], eye_d[:])
            nc.sync.dma_start(rotidx[:], rotidx_d[:])
            nc.vector.memset(bexp[:], -1.0 / T)
            nc.vector.memset(ones_bf[:], 1.0)
            nc.vector.memset(es[:], 0.0)
            for ch in range(NZ):
                nc.vector.memset(z_rows[:, ch * 129 + 128:ch * 129 + 129], 1.0)

            # ---- phase 1: normalize + transpose + split AllGather ----
            with tc.tile_pool(name="ps_pre", bufs=2, space="PSUM") as psp:
                for h in range(2):
                    for ch in range(h * 8, h * 8 + 8):
                        if ch < NCH:
                            src = rows_a[:, ch * 257:ch * 257 + 128]
                        else:
                            src = rows_b[:, (ch - NCH) * 129:(ch - NCH) * 129 + 128]
                        sq_s = sp.tile([128, 128], F32, name="sq_s", tag="sq_s")
                        nc.scalar.activation(sq_s[:], src, AF.Square,
                                             accum_out=nsq_all[:, ch:ch + 1])
                    h8 = h * 8
                    nc.scalar.activation(lnq[:, h8:h8 + 8], nsq_all[:, h8:h8 + 8], AF.Ln)
                    nc.scalar.activation(rinv_all[:, h8:h8 + 8], lnq[:, h8:h8 + 8],
                                         AF.Exp, scale=-0.5)
                    for ch in range(h * 8, h * 8 + 8):
                        if ch < NCH:
                            src = rows_a[:, ch * 257:ch * 257 + 128]
                        else:
                            src = rows_b[:, (ch - NCH) * 129:(ch - NCH) * 129 + 128]
                        zsl = z_rows[:, ch * 129:ch * 129 + 128]
                        nc.vector.tensor_scalar_mul(zsl, src, rinv_all[:, ch:ch + 1])
                        tr_ps = psp.tile([128, 128], F32, name="tr_ps", tag="tr_ps")
                        nc.tensor.transpose(tr_ps[:], zsl, eye[:])
                        nc.vector.tensor_copy(featT[:, ch * 128:(ch + 1) * 128], tr_ps[:])
                    if h == 0:
                        nc.sync.dma_start(ag1_in[:], featT[:, 0:1024])
                        nc.gpsimd.collective_compute(
                            "AllGather", OP.bypass, replica_groups=[list(range(NC))],
                            ins=[ag1_in[:]], outs=[ag1_out[:]],
                        )
                    else:
                        nc.sync.dma_start(ag2_in[:], featT[:, 1024:2048])
                        nc.gpsimd.collective_compute(
                            "AllGather", OP.bypass, replica_groups=[list(range(NC))],
                            ins=[ag2_in[:]], outs=[ag2_out[:]],
                        )

                # deferred: exact bf16 self-dots (DVE, fills the AG window)
                for ch in range(NZ):
                    zbf = sp.tile([128, 128], BF16, name="zbf", tag="zbf")
                    nc.vector.tensor_copy(zbf[:], z_rows[:, ch * 129:ch * 129 + 128])
                    zb2 = sp.tile([128, 128], F32, name="zb2", tag="zb2")
                    nc.vector.tensor_mul(zb2[:], zbf[:], zbf[:])
                    nc.vector.tensor_reduce(sd_all[:, ch:ch + 1], zb2[:],
                                            AX.X, OP.add)

            # ---- band A: l = 0..15 (featR-free intra tiles) ----
            with tc.tile_pool(name="ps_bandA", bufs=2, space="PSUM") as psba:
                for l in range(16):
                    _band_iter(nc, psba, ep, sp, featT, featR, es, s_oth, bexp, l)
            # xsq: one wide square + DVE reduces (fills the AG window)
            xsq_scr = pp.tile([128, SH], F32, name="xsq_scr")
            nc.scalar.activation(xsq_scr[:], eb_sb[:], AF.Square)
            for ch in range(NCH):
                nc.vector.tensor_reduce(xsq_b[:, ch:ch + 1],
                                        xsq_scr[:, ch * 128:(ch + 1) * 128],
                                        AX.X, OP.add)

            # ---- phase 2: assignment + one-hots + seg sums + seg AllReduce ----
            with tc.tile_pool(name="ps_ph2", bufs=2, space="PSUM") as psp2:
                for ch in range(NZ):
                    sc_ps = psp2.tile([128, K], F32, name="sc_ps", tag="sc_ps")
                    nc.tensor.matmul(sc_ps[:], encT_sb[:, ch * 128:(ch + 1) * 128], csc[:])
                    s_sb = sp.tile([128, K], F32, name="s_sb", tag="s_sb")
                    nc.vector.scalar_tensor_tensor(s_sb[:], sc_ps[:], 1.0, csq[:],
                                                   op0=OP.mult, op1=OP.add)
                    mn = sp.tile([128, 1], F32, name="mn", tag="mn")
                    nc.vector.tensor_reduce(mn[:], s_sb[:], AX.X, OP.min)
                    tmp = sp.tile([128, K], F32, name="tmp", tag="tmp")
                    nc.vector.scalar_tensor_tensor(tmp[:], s_sb[:], mn[:], i64m[:],
                                                   op0=OP.is_equal, op1=OP.mult)
                    nc.vector.tensor_reduce(idxm_all[:, ch:ch + 1], tmp[:], AX.X, OP.min)
                    nc.vector.tensor_scalar(cl_oh[:, ch * K:(ch + 1) * K], i64m[:],
                                            idxm_all[:, ch:ch + 1], None, op0=OP.is_equal)
                with tc.tile_pool(name="ps_seg", bufs=1, space="PSUM") as pseg:
                    seg_a = pseg.tile([64, 257], F32, name="seg_a")
                    seg_b = pseg.tile([64, 129], F32, name="seg_b")
                    p_ps = pseg.tile([C, 129], F32, name="p_ps")
                    for ch in range(NCH):
                        nc.tensor.matmul(seg_a[:], cl_oh[:, ch * K:(ch + 1) * K],
                                         rows_a[:, ch * 257:(ch + 1) * 257],
                                         start=(ch == 0), stop=(ch == NCH - 1))
                    for ch in range(NCH):
                        oh = cl_oh[:, (NCH + ch) * K:(NCH + ch + 1) * K]
                        nc.tensor.matmul(seg_b[:], oh, rows_b[:, ch * 129:(ch + 1) * 129],
                                         start=(ch == 0), stop=(ch == NCH - 1))
                    for ch in range(NZ):
                        loh = sp.tile([128, C], F32, name="loh", tag="loh")
                        nc.vector.tensor_scalar(loh[:], i10r[:],
                                                labf[:, (ch % NCH):(ch % NCH) + 1],
                                                None, op0=OP.is_equal)
                        nc.tensor.matmul(p_ps[:], loh[:], z_rows[:, ch * 129:(ch + 1) * 129],
                                         start=(ch == 0), stop=(ch == NZ - 1))
                    st_a = sp.tile([64, 257], F32, name="st_a", tag="st_a")
                    nc.vector.tensor_copy(st_a[:], seg_a[:])
                    st_b = sp.tile([64, 129], F32, name="st_b", tag="st_b")
                    nc.vector.tensor_copy(st_b[:], seg_b[:])
                    st_p = sp.tile([C, 129], F32, name="st_p", tag="st_p")
                    nc.vector.tensor_copy(st_p[:], p_ps[:])
                    nc.sync.dma_start(ar_in[0:C, 0:129], st_p[:])
                    nc.sync.dma_start(ar_in[0:64, 129:386], st_a[:])
                    nc.sync.dma_start(ar_in[0:64, 386:515], st_b[:])
            # ---- featR: rotated gather of cores r+1..r+4 from the two AGs ----
            for q in range(1, 5):
                base = (q - 1) * 2048
                nc.gpsimd.indirect_dma_start(
                    out=featR[:, base:base + 1024], out_offset=None,
                    in_=ag1_out[:],
                    in_offset=bass.IndirectOffsetOnAxis(ap=rotidx[:, q - 1:q], axis=0),
                )
                nc.gpsimd.indirect_dma_start(
                    out=featR[:, base + 1024:base + 2048], out_offset=None,
                    in_=ag2_out[:],
                    in_offset=bass.IndirectOffsetOnAxis(ap=rotidx[:, q - 1:q], axis=0),
                )
            nc.gpsimd.collective_compute(
                "AllReduce", OP.add, replica_groups=[list(range(NC))],
                ins=[ar_in[:]], outs=[ar_out[:]],
            )
            nc.sync.dma_start(seg_a_sb[:], ar_out[0:64, 129:386])
            nc.sync.dma_start(seg_b_sb[:], ar_out[0:64, 386:515])
            p_stage = pp.tile([C, 129], F32, name="p_stage")
            nc.sync.dma_start(p_stage[:], ar_out[0:C, 0:129])
            nc.sync.dma_start(out2_d[:, 0:257], seg_a_sb[:])
            nc.sync.dma_start(out2_d[:, 257:386], seg_b_sb[:])
            nc.sync.dma_start(out2_d[0:C, 386:515], p_stage[:])

            # ---- band B: l = 16..79 + in-pool tail ----
            with tc.tile_pool(name="ps_band", bufs=2, space="PSUM") as psb:
                for l in range(16, LBAND):
                    _band_iter(nc, psb, ep, sp, featT, featR, es, s_oth, bexp, l)
                    if l == 60:
                        # support prototypes (DVE only; seg AR long done)
                        ca = sp.tile([64, 1], F32, name="ca", tag="ca")
                        nc.vector.tensor_scalar_max(ca[:], seg_a_sb[:, 256:257], 1.0)
                        rac = sp.tile([64, 1], F32, name="rac", tag="rac")
                        nc.vector.reciprocal(rac[:], ca[:])
                        nc.vector.tensor_scalar_mul(sup[:], seg_a_sb[:, 128:256],
                                                    rac[:])
                # tail matmuls reuse the band psum buffers
                psA = psb.tile([128, 2048], F32, name="ps_f", tag="ps_f")
                nc.tensor.transpose(psA[:, 0:64], sup[:], eye[0:64, 0:64])
                nc.vector.tensor_scalar_mul(supT2[:], psA[:, 0:64], -2.0)
                psB = psb.tile([128, 2048], F32, name="ps_f", tag="ps_f")
                for ch in range(NCH):
                    nc.tensor.matmul(psB[:, ch * K:(ch + 1) * K],
                                     encT_sb[:, (NCH + ch) * 128:(NCH + ch + 1) * 128],
                                     supT2[:])
                for t in range(16):
                    nc.tensor.matmul(psB[:, 512 + t:513 + t],
                                     es[:, t * 128:(t + 1) * 128], ones_bf[:])
                nc.vector.tensor_copy(dn_sb[:], psB[:, 0:512])
                nc.vector.tensor_copy(sm_sb[:], psB[:, 512:528])

            # ---- ship results ----
            nc.sync.dma_start(out_d[:, 0:512], dn_sb[:])
            nc.sync.dma_start(out_d[:, 512:592], s_oth[:])
            nc.sync.dma_start(out_d[:, 592:608], sm_sb[:])
            nc.sync.dma_start(out_d[:, 640:656], sd_all[:])
            nc.sync.dma_start(out_d[:, 656:664], xsq_b[:])
            nc.sync.dma_start(out_d[:, 664:680], idxm_all[:])

    nc.compile()
    return nc


def _prep_inputs(encodings_a, encodings_b, projections_a, projections_b,
                 cluster_centers, labels):
    ea = np.ascontiguousarray(encodings_a, dtype=np.float32)
    eb = np.ascontiguousarray(encodings_b, dtype=np.float32)
    pa = np.ascontiguousarray(projections_a, dtype=np.float32)
    pb = np.ascontiguousarray(projections_b, dtype=np.float32)
    cc = np.ascontiguousarray(cluster_centers, dtype=np.float32)
    lab = np.asarray(labels).astype(np.float32)

    csc = np.ascontiguousarray((-2.0 * cc).T)
    csq = np.tile(np.sum(cc * cc, axis=1)[None, :], (128, 1)).astype(np.float32)
    i10r = np.tile(np.arange(C, dtype=np.float32)[None, :], (128, 1))
    i64m = np.tile((np.arange(K, dtype=np.float32) - K)[None, :], (128, 1))
    eye = np.eye(128, dtype=np.float32)

    _CACHE["aux"] = {"pa": pa, "pb": pb, "lab": lab.astype(np.int64)}
    in_maps = []
    for r in range(NC):
        s = slice(r * SH, (r + 1) * SH)
        labs = lab[s]
        ri = np.stack([((r + 1 + j) % NC) * 128 + np.arange(128) for j in range(4)],
                      axis=1).astype(np.int32)
        labc = labs.reshape(NCH, 128).T.astype(np.float32)   # [128, 8]
        in_maps.append({
            "pa": pa[s], "pb": pb[s], "ea": ea[s], "eb": eb[s],
            "encT": np.ascontiguousarray(np.concatenate([ea[s], eb[s]], 0).T),
            "centers_sc": csc, "csq": csq,
            "labels_f": np.ascontiguousarray(labc),
            "iota10r": i10r, "iota64m": i64m, "eye": eye,
            "rotidx": ri,
        })
    return in_maps


def _combine(results):
    outs = [np.asarray(res["out"], dtype=np.float64) for res in results]
    seg = np.asarray(results[0]["out2"], dtype=np.float64)

    # ---- l_main: scatter row-sum partials, assemble log-denominators ----
    aux = _CACHE["aux"]
    P = seg[0:C, 386:515]                    # class sums [10, 129]
    S_glob = np.zeros((128, 128))  # [block, row-in-block]
    for r in range(NC):
        s_oth = outs[r][:, 512:592]          # [128(p), 80(l)]
        g = (16 * r + np.arange(LBAND)) % 128
        np.add.at(S_glob, g, s_oth.T)
    l_main_sum = 0.0
    for r in range(NC):
        s = slice(r * SH, (r + 1) * SH)
        za = aux["pa"][s] / np.linalg.norm(aux["pa"][s], axis=1, keepdims=True)
        zb = aux["pb"][s] / np.linalg.norm(aux["pb"][s], axis=1, keepdims=True)
        zr = np.concatenate([za, zb], 0).astype(np.float64)       # [2048, 128]
        labs2 = np.concatenate([aux["lab"][s], aux["lab"][s]])
        a = np.sum(zr * P[labs2, 0:128], axis=1).reshape(NZ, 128).T
        n2 = P[labs2, 128].reshape(NZ, 128).T
        sm = outs[r][:, 592:608]             # [128, 16] col-side sums (my rows)
        sd = outs[r][:, 640:656]
        S = S_glob[16 * r:16 * r + 16].T + sm - np.exp((sd - 1.0) / T)
        m1 = (a - n2) / T / (n2 - 1.0)
        l_main_sum += np.sum(m1 - np.log(S))
    l_main = -(T / BT) * l_main_sum / (2 * B)

    # ---- l_p: prototype NTXent on host (64-dim, trivial) ----
    ca = np.maximum(seg[:, 256], 1.0)
    cb = np.maximum(seg[:, 385], 1.0)
    proto_a = seg[:, 0:128] / ca[:, None]
    proto_b = seg[:, 257:385] / cb[:, None]
    za = proto_a / np.linalg.norm(proto_a, axis=1, keepdims=True)
    zb = proto_b / np.linalg.norm(proto_b, axis=1, keepdims=True)
    z = np.concatenate([za, zb], 0)
    n = 2 * K
    sim = (z @ z.T) / T
    np.fill_diagonal(sim, -np.inf)
    pos = (np.arange(n) + K) % n
    mx = np.max(sim, axis=1, keepdims=True)
    logp = sim - mx - np.log(np.sum(np.exp(sim - mx), axis=1, keepdims=True))
    l_p = -np.mean(logp[np.arange(n), pos])

    # ---- l_n: meta CE from shipped -2*e.sup logits ----
    sup_v = seg[:, 128:256] / ca[:, None]
    ssq = np.sum(sup_v * sup_v, axis=1)
    l_n_sum = 0.0
    for r in range(NC):
        dn = outs[r][:, 0:512].reshape(128, NCH, K)
        xsq = outs[r][:, 656:664]
        lb = (outs[r][:, 664:680] + K)[:, NCH:].astype(np.int64)  # [128, 8]
        d2 = dn + xsq[:, :, None] + ssq[None, None, :]
        dd = np.sqrt(np.maximum(d2, 0.0))
        mxd = np.min(dd, axis=2, keepdims=True)
        ls = -(dd - mxd) - np.log(np.sum(np.exp(-(dd - mxd)), axis=2, keepdims=True))
        p_idx, c_idx = np.meshgrid(np.arange(128), np.arange(NCH), indexing="ij")
        l_n_sum += np.sum(ls[p_idx, c_idx, lb])
    l_n = -l_n_sum / B

    return np.float32(l_main + W_P * l_p + W_N * l_n)


def kernel(encodings_a, encodings_b, projections_a, projections_b,
           cluster_centers, labels):
    if "nc" not in _CACHE:
        _CACHE["nc"] = _build()
    nc = _CACHE["nc"]
    in_maps = _prep_inputs(encodings_a, encodings_b, projections_a,
                           projections_b, cluster_centers, labels)
    res = bass_utils.run_bass_kernel_spmd(nc, in_maps, core_ids=list(range(NC)))
    return _combine(res.results)


# revision 32
# speedup vs baseline: 1.2102x; 1.1606x over previous
"""CalibreLoss TRN2 kernel v2: Act-queue-bound symmetric half-band SupCon.

Data-parallel over batch B across 8 cores. vs v1:
  * diag (d=0) and ring (d=64) tiles are folded into the 80-iteration band
    loop (one EXP activation per l, accum_out = row sums), cutting Act
    instruction count 120 -> 80 and removing separate s_d/s_r paths.
  * e_t / es are bf16 -> DVE es adds run in 2x mode (~half the time).
  * a 4-byte dummy collective issued first absorbs the CC-ring entry
    barrier; the feature AllGather is split in two halves so band l=16
    can start as soon as the first half lands.
  * the segment-sum AllReduce carries only [64,516] and runs during the
    band; row-sum partials (s_oth), colsums (sm), class dots (a), meta-CE
    logits (dn) etc. are shipped raw to the host, which does the final
    scatter/ln/softmax/NTXent assembly in numpy. No second AllReduce.
  * single act table set (exp/ln); sqrt/rsqrt eliminated everywhere.
"""

import sys

sys.path.insert(0, "/opt/trn_rl_repo")

import numpy as np

import concourse.bass as bass
import concourse.bacc as bacc
import concourse.mybir as mybir
import concourse.tile as tile
from concourse import bass_utils

F32 = mybir.dt.float32
BF16 = mybir.dt.bfloat16
I32 = mybir.dt.int32
AX = mybir.AxisListType
OP = mybir.AluOpType
AF = mybir.ActivationFunctionType

B = 8192
D = 128
K = 64
C = 10
T = 0.07
BT = 0.07
W_P = 0.5
W_N = 0.5
NC = 8
SH = B // NC          # 1024 rows of each input per core
NCH = SH // 128       # 8 chunks per input
NZ = 2 * NCH          # 16 z chunks (proj_a + proj_b)
LBAND = 80            # band iterations: l = 0..79 (rot blocks)
OUTW = 680
DUMMY_CC = False
PRS_GATHER = True
USE_TTR = False

_CACHE = {}


def _band_iter(nc, psb, ep, sp, featT, featR, es, s_oth, bexp, l):
    if l <= 15:
        wlo, whi = 0, l
        lhs = featT[:, l * 128:(l + 1) * 128]
    elif l <= 63:
        wlo, whi = 0, 15
        lhs = featR[:, (l - 16) * 128:(l - 15) * 128]
    else:
        wlo, whi = l - 64, 15
        lhs = featR[:, (l - 16) * 128:(l - 15) * 128]
    wid = (whi - wlo + 1) * 128
    psw = 1024 if l >= 72 else 2048
    ps = psb.tile([128, psw], F32, name="ps_f", tag="ps_f")
    c0 = wlo * 128
    cend = (whi + 1) * 128
    while c0 < cend:
        n = min(512, cend - c0)
        nc.tensor.matmul(ps[:, c0 - wlo * 128:c0 - wlo * 128 + n],
                         lhs, featT[:, c0:c0 + n])
        c0 += n
    e_t = ep.tile([128, 2048], BF16, name="e_t", tag="e_t")
    nc.scalar.activation(e_t[:, 0:wid], ps[:, 0:wid], AF.Exp,
                         scale=1.0 / T, bias=bexp[:],
                         accum_out=s_oth[:, l:l + 1])
    if 1 <= l <= 63:
        # skip trailing diag block for l<=15
        ew = (min(whi, l - 1) - wlo + 1) * 128
        nc.vector.tensor_add(es[:, wlo * 128:wlo * 128 + ew],
                             es[:, wlo * 128:wlo * 128 + ew],
                             e_t[:, 0:ew])
    elif l >= 64 and l < 79:
        # skip leading ring block
        ew = (whi - wlo) * 128
        nc.vector.tensor_add(es[:, (wlo + 1) * 128:(wlo + 1) * 128 + ew],
                             es[:, (wlo + 1) * 128:(wlo + 1) * 128 + ew],
                             e_t[:, 128:128 + ew])


def _build():
    nc = bacc.Bacc("TRN2", target_bir_lowering=False, debug=False, num_devices=NC)

    pa_d = nc.dram_tensor("pa", [SH, 128], F32, kind="ExternalInput")
    pb_d = nc.dram_tensor("pb", [SH, 128], F32, kind="ExternalInput")
    ea_d = nc.dram_tensor("ea", [SH, 128], F32, kind="ExternalInput")
    eb_d = nc.dram_tensor("eb", [SH, 128], F32, kind="ExternalInput")
    encT_d = nc.dram_tensor("encT", [128, 2048], F32, kind="ExternalInput")
    csc_d = nc.dram_tensor("centers_sc", [128, K], F32, kind="ExternalInput")
    csq_d = nc.dram_tensor("csq", [128, K], F32, kind="ExternalInput")
    labf_d = nc.dram_tensor("labels_f", [128, NCH], F32, kind="ExternalInput")
    i10r_d = nc.dram_tensor("iota10r", [128, C], F32, kind="ExternalInput")
    i64m_d = nc.dram_tensor("iota64m", [128, K], F32, kind="ExternalInput")
    eye_d = nc.dram_tensor("eye", [128, 128], F32, kind="ExternalInput")
    rotidx_d = nc.dram_tensor("rotidx", [128, 4], I32, kind="ExternalInput")

    out_d = nc.dram_tensor("out", [128, OUTW], F32, kind="ExternalOutput")
    out2_d = nc.dram_tensor("out2", [64, 515], F32, kind="ExternalOutput")

    with tile.TileContext(nc) as tc:
        with (
            tc.tile_pool(name="persist", bufs=1) as pp,
            tc.tile_pool(name="scratch", bufs=4) as sp,
            tc.tile_pool(name="etp", bufs=6) as ep,
            tc.tile_pool(name="dram", bufs=1, space="DRAM") as dp,
        ):
            featT = pp.tile([128, 2048], BF16, name="featT")
            featR = pp.tile([128, 8192], BF16, name="featR")
            z_rows = pp.tile([128, NZ * 129], F32, name="z_rows")
            rows_a = pp.tile([128, NCH * 257], F32, name="rows_a")
            rows_b = pp.tile([128, NCH * 129], F32, name="rows_b")
            eb_sb = pp.tile([128, SH], F32, name="eb_sb")
            encT_sb = pp.tile([128, 2048], F32, name="encT_sb")
            nsq_all = pp.tile([128, NZ], F32, name="nsq_all")
            lnq = pp.tile([128, NZ], F32, name="lnq")
            rinv_all = pp.tile([128, NZ], F32, name="rinv_all")
            sd_all = pp.tile([128, NZ], F32, name="sd_all")
            xsq_b = pp.tile([128, NCH], F32, name="xsq_b")
            idxm_all = pp.tile([128, NZ], F32, name="idxm_all")
            cl_oh = pp.tile([128, NZ * K], F32, name="cl_oh")
            s_oth = pp.tile([128, LBAND], F32, name="s_oth")
            sm_sb = pp.tile([128, 16], F32, name="sm_sb")
            es = pp.tile([128, 2048], BF16, name="es")
            dn_sb = pp.tile([128, NCH * K], F32, name="dn_sb")
            ebT_bf = pp.tile([128, SH], BF16, name="ebT_bf")
            supT2_bf = pp.tile([128, K], BF16, name="supT2_bf")
            seg_a_sb = pp.tile([64, 257], F32, name="seg_a_sb")
            sup = pp.tile([64, 128], F32, name="sup")
            supT2 = pp.tile([128, 64], F32, name="supT2")
            csc = pp.tile([128, K], F32, name="csc_sb")
            csq = pp.tile([128, K], F32, name="csq_sb")
            labf = pp.tile([128, NCH], F32, name="labf_sb")
            i10r = pp.tile([128, C], F32, name="i10r_sb")
            i64m = pp.tile([128, K], F32, name="i64m_sb")
            eye = pp.tile([128, 128], F32, name="eye_sb")
            ones_bf = pp.tile([128, 1], BF16, name="ones_bf")
            bexp = pp.tile([128, 1], F32, name="bexp")
            rotidx = pp.tile([128, 4], I32, name="rotidx_sb")
            seg_b_sb = pp.tile([64, 129], F32, name="seg_b_sb")
            if DUMMY_CC:
                dm_sb = pp.tile([64, 129], F32, name="dm_sb")
                d_in = dp.tile([64, 129], F32, name="d_in")
                d_out = dp.tile([64, 129], F32, name="d_out", addr_space="Shared")
            ag1_in = dp.tile([128, 1024], BF16, name="ag1_in")
            ag1_out = dp.tile([128 * NC, 1024], BF16, name="ag1_out", addr_space="Shared")
            ag2_in = dp.tile([128, 1024], BF16, name="ag2_in")
            ag2_out = dp.tile([128 * NC, 1024], BF16, name="ag2_out", addr_space="Shared")
            ar_in = dp.tile([64, 516], F32, name="ar_in")
            ar_out = dp.tile([64, 516], F32, name="ar_out", addr_space="Shared")

            # ---- dummy collective first: absorbs CC entry barrier ----
            if DUMMY_CC:
                nc.vector.memset(dm_sb[:], 0.0)
                nc.sync.dma_start(d_in[:], dm_sb[:])
                nc.gpsimd.collective_compute(
                    "AllReduce", OP.add, replica_groups=[list(range(NC))],
                    ins=[d_in[:]], outs=[d_out[:]],
                )

            # ---- input loads (pa/pb per-chunk so phase1 pipelines with DMA) ----
            ra3 = rows_a[:].rearrange("p (c w) -> p c w", c=NCH)
            eb3 = eb_sb[:].rearrange("p (c w) -> p c w", c=NCH)
            for ch in range(NCH):
                nc.sync.dma_start(rows_a[:, ch * 257:ch * 257 + 128],
                                  pa_d[ch * 128:(ch + 1) * 128, :])
            for ch in range(NCH):
                nc.sync.dma_start(rows_b[:, ch * 129:ch * 129 + 128],
                                  pb_d[ch * 128:(ch + 1) * 128, :])
            for ch in range(NCH):
                nc.vector.memset(rows_a[:, ch * 257 + 256:ch * 257 + 257], 1.0)
                nc.vector.memset(rows_b[:, ch * 129 + 128:ch * 129 + 129], 1.0)
            nc.sync.dma_start(csc[:], csc_d[:])
            nc.sync.dma_start(csq[:], csq_d[:])
            nc.sync.dma_start(labf[:], labf_d[:])
            nc.sync.dma_start(i10r[:], i10r_d[:])
            nc.sync.dma_start(i64m[:], i64m_d[:])
            nc.sync.dma_start(eye[:], eye_d[:])
            nc.sync.dma_start(rotidx[:], rotidx_d[:])
            nc.vector.memset(bexp[:], -1.0 / T)
            nc.vector.memset(ones_bf[:], 1.0)
            nc.vector.memset(es[:], 0.0)
            for ch in range(NZ):
                nc.vector.memset(z_rows[:, ch * 129 + 128:ch * 129 + 129], 1.0)

            # ---- phase 1: normalize + transpose + split AllGather ----
            with tc.tile_pool(name="ps_pre", bufs=2, space="PSUM") as psp:
                for h in range(2):
                    for ch in range(h * 8, h * 8 + 8):
                        if ch < NCH:
                            src = rows_a[:, ch * 257:ch * 257 + 128]
                        else:
                            src = rows_b[:, (ch - NCH) * 129:(ch - NCH) * 129 + 128]
                        sq_s = sp.tile([128, 128], F32, name="sq_s", tag="sq_s")
                        nc.scalar.activation(sq_s[:], src, AF.Square,
                                             accum_out=nsq_all[:, ch:ch + 1])
                    h8 = h * 8
                    nc.scalar.activation(lnq[:, h8:h8 + 8], nsq_all[:, h8:h8 + 8], AF.Ln)
                    nc.scalar.activation(rinv_all[:, h8:h8 + 8], lnq[:, h8:h8 + 8],
                                         AF.Exp, scale=-0.5)
                    for ch in range(h * 8, h * 8 + 8):
                        if ch < NCH:
                            src = rows_a[:, ch * 257:ch * 257 + 128]
                        else:
                            src = rows_b[:, (ch - NCH) * 129:(ch - NCH) * 129 + 128]
                        zsl = z_rows[:, ch * 129:ch * 129 + 128]
                        nc.vector.tensor_scalar_mul(zsl, src, rinv_all[:, ch:ch + 1])
                        tr_ps = psp.tile([128, 128], F32, name="tr_ps", tag="tr_ps")
                        nc.tensor.transpose(tr_ps[:], zsl, eye[:])
                        nc.vector.tensor_copy(featT[:, ch * 128:(ch + 1) * 128], tr_ps[:])
                    if h == 0:
                        nc.sync.dma_start(ag1_in[:], featT[:, 0:1024])
                        nc.gpsimd.collective_compute(
                            "AllGather", OP.bypass, replica_groups=[list(range(NC))],
                            ins=[ag1_in[:]], outs=[ag1_out[:]],
                        )
                    else:
                        nc.sync.dma_start(ag2_in[:], featT[:, 1024:2048])
                        nc.gpsimd.collective_compute(
                            "AllGather", OP.bypass, replica_groups=[list(range(NC))],
                            ins=[ag2_in[:]], outs=[ag2_out[:]],
                        )

            # big loads issued after the AG input DMAs (less DMA contention)
            nc.sync.dma_start(ra3[:, :, 128:256],
                              ea_d[:].rearrange("(c p) d -> p c d", c=NCH))
            nc.sync.dma_start(eb3[:, :, :],
                              eb_d[:].rearrange("(c p) d -> p c d", c=NCH))
            nc.sync.dma_start(encT_sb[:], encT_d[:])

            # ---- band A: l = 0..15 (featR-free intra tiles) ----
            with tc.tile_pool(name="ps_bandA", bufs=2, space="PSUM") as psba:
                for l in range(16):
                    _band_iter(nc, psba, ep, sp, featT, featR, es, s_oth, bexp, l)
            # deferred: exact bf16 self-dots (DVE, fills the AG window)
            for ch in range(NZ):
                zbf = sp.tile([128, 128], BF16, name="zbf", tag="zbf")
                nc.vector.tensor_copy(zbf[:], z_rows[:, ch * 129:ch * 129 + 128])
                zb2 = sp.tile([128, 128], F32, name="zb2", tag="zb2")
                nc.vector.tensor_mul(zb2[:], zbf[:], zbf[:])
                nc.vector.tensor_reduce(sd_all[:, ch:ch + 1], zb2[:],
                                        AX.X, OP.add)
            # xsq: one wide square + DVE reduces (fills the AG window)
            xsq_scr = pp.tile([128, SH], F32, name="xsq_scr")
            nc.scalar.activation(xsq_scr[:], eb_sb[:], AF.Square)
            for ch in range(NCH):
                nc.vector.tensor_reduce(xsq_b[:, ch:ch + 1],
                                        xsq_scr[:, ch * 128:(ch + 1) * 128],
                                        AX.X, OP.add)

            # ---- phase 2: assignment + one-hots + seg sums + seg AllReduce ----
            with tc.tile_pool(name="ps_ph2", bufs=2, space="PSUM") as psp2:
                for ch in range(NZ):
                    sc_ps = psp2.tile([128, K], F32, name="sc_ps", tag="sc_ps")
                    nc.tensor.matmul(sc_ps[:], encT_sb[:, ch * 128:(ch + 1) * 128], csc[:])
                    s_sb = sp.tile([128, K], F32, name="s_sb", tag="s_sb")
                    nc.vector.scalar_tensor_tensor(s_sb[:], sc_ps[:], 1.0, csq[:],
                                                   op0=OP.mult, op1=OP.add)
                    mn = sp.tile([128, 1], F32, name="mn", tag="mn")
                    nc.vector.tensor_reduce(mn[:], s_sb[:], AX.X, OP.min)
                    tmp = sp.tile([128, K], F32, name="tmp", tag="tmp")
                    nc.vector.scalar_tensor_tensor(tmp[:], s_sb[:], mn[:], i64m[:],
                                                   op0=OP.is_equal, op1=OP.mult)
                    nc.vector.tensor_reduce(idxm_all[:, ch:ch + 1], tmp[:], AX.X, OP.min)
                    nc.vector.tensor_scalar(cl_oh[:, ch * K:(ch + 1) * K], i64m[:],
                                            idxm_all[:, ch:ch + 1], None, op0=OP.is_equal)
                with tc.tile_pool(name="ps_seg", bufs=1, space="PSUM") as pseg:
                    seg_a = pseg.tile([64, 257], F32, name="seg_a")
                    seg_b = pseg.tile([64, 129], F32, name="seg_b")
                    p_ps = pseg.tile([C, 129], F32, name="p_ps")
                    for ch in range(NCH):
                        nc.tensor.matmul(seg_a[:], cl_oh[:, ch * K:(ch + 1) * K],
                                         rows_a[:, ch * 257:(ch + 1) * 257],
                                         start=(ch == 0), stop=(ch == NCH - 1))
                    for ch in range(NCH):
                        oh = cl_oh[:, (NCH + ch) * K:(NCH + ch + 1) * K]
                        nc.tensor.matmul(seg_b[:], oh, rows_b[:, ch * 129:(ch + 1) * 129],
                                         start=(ch == 0), stop=(ch == NCH - 1))
                    for ch in range(NZ):
                        loh = sp.tile([128, C], F32, name="loh", tag="loh")
                        nc.vector.tensor_scalar(loh[:], i10r[:],
                                                labf[:, (ch % NCH):(ch % NCH) + 1],
                                                None, op0=OP.is_equal)
                        nc.tensor.matmul(p_ps[:], loh[:], z_rows[:, ch * 129:(ch + 1) * 129],
                                         start=(ch == 0), stop=(ch == NZ - 1))
                    st_a = sp.tile([64, 257], F32, name="st_a", tag="st_a")
                    nc.vector.tensor_copy(st_a[:], seg_a[:])
                    st_b = sp.tile([64, 129], F32, name="st_b", tag="st_b")
                    nc.vector.tensor_copy(st_b[:], seg_b[:])
                    st_p = sp.tile([C, 129], F32, name="st_p", tag="st_p")
                    nc.vector.tensor_copy(st_p[:], p_ps[:])
                    nc.sync.dma_start(ar_in[0:C, 0:129], st_p[:])
                    nc.sync.dma_start(ar_in[0:64, 129:386], st_a[:])
                    nc.sync.dma_start(ar_in[0:64, 386:515], st_b[:])
            # ---- featR: rotated gather of cores r+1..r+4 from the two AGs ----
            for q in range(1, 5):
                base = (q - 1) * 2048
                nc.gpsimd.indirect_dma_start(
                    out=featR[:, base:base + 1024], out_offset=None,
                    in_=ag1_out[:],
                    in_offset=bass.IndirectOffsetOnAxis(ap=rotidx[:, q - 1:q], axis=0),
                )
                nc.gpsimd.indirect_dma_start(
                    out=featR[:, base + 1024:base + 2048], out_offset=None,
                    in_=ag2_out[:],
                    in_offset=bass.IndirectOffsetOnAxis(ap=rotidx[:, q - 1:q], axis=0),
                )
            nc.gpsimd.collective_compute(
                "AllReduce", OP.add, replica_groups=[list(range(NC))],
                ins=[ar_in[:]], outs=[ar_out[:]],
            )
            nc.sync.dma_start(seg_a_sb[:], ar_out[0:64, 129:386])
            nc.sync.dma_start(seg_b_sb[:], ar_out[0:64, 386:515])
            p_stage = pp.tile([C, 129], F32, name="p_stage")
            nc.sync.dma_start(p_stage[:], ar_out[0:C, 0:129])
            nc.sync.dma_start(out2_d[:, 0:257], seg_a_sb[:])
            nc.sync.dma_start(out2_d[:, 257:386], seg_b_sb[:])
            nc.sync.dma_start(out2_d[0:C, 386:515], p_stage[:])

            # ---- band B: l = 16..79 + in-pool tail ----
            with tc.tile_pool(name="ps_band", bufs=2, space="PSUM") as psb:
                for l in range(16, LBAND):
                    _band_iter(nc, psb, ep, sp, featT, featR, es, s_oth, bexp, l)
                    if l == 60:
                        # support prototypes (DVE only; seg AR long done)
                        ca = sp.tile([64, 1], F32, name="ca", tag="ca")
                        nc.vector.tensor_scalar_max(ca[:], seg_a_sb[:, 256:257], 1.0)
                        rac = sp.tile([64, 1], F32, name="rac", tag="rac")
                        nc.vector.reciprocal(rac[:], ca[:])
                        nc.vector.tensor_scalar_mul(sup[:], seg_a_sb[:, 128:256],
                                                    rac[:])
                        nc.vector.tensor_copy(ebT_bf[:], encT_sb[:, SH:2 * SH])
                # tail matmuls reuse the band psum buffers
                psA = psb.tile([128, 2048], F32, name="ps_f", tag="ps_f")
                nc.tensor.transpose(psA[:, 0:64], sup[:], eye[0:64, 0:64])
                nc.vector.tensor_scalar_mul(supT2_bf[:], psA[:, 0:64], -2.0)
                psB = psb.tile([128, 2048], F32, name="ps_f", tag="ps_f")
                for ch in range(NCH):
                    nc.tensor.matmul(psB[:, ch * K:(ch + 1) * K],
                                     ebT_bf[:, ch * 128:(ch + 1) * 128],
                                     supT2_bf[:])
                for t in range(16):
                    nc.tensor.matmul(psB[:, 512 + t:513 + t],
                                     es[:, t * 128:(t + 1) * 128], ones_bf[:])
                nc.vector.tensor_copy(dn_sb[:], psB[:, 0:512])
                nc.vector.tensor_copy(sm_sb[:], psB[:, 512:528])

            # ---- ship results ----
            nc.sync.dma_start(out_d[:, 0:512], dn_sb[:])
            nc.sync.dma_start(out_d[:, 512:592], s_oth[:])
            nc.sync.dma_start(out_d[:, 592:608], sm_sb[:])
            nc.sync.dma_start(out_d[:, 640:656], sd_all[:])
            nc.sync.dma_start(out_d[:, 656:664], xsq_b[:])
            nc.sync.dma_start(out_d[:, 664:680], idxm_all[:])

    nc.compile()
    return nc


def _prep_inputs(encodings_a, encodings_b, projections_a, projections_b,
                 cluster_centers, labels):
    ea = np.ascontiguousarray(encodings_a, dtype=np.float32)
    eb = np.ascontiguousarray(encodings_b, dtype=np.float32)
    pa = np.ascontiguousarray(projections_a, dtype=np.float32)
    pb = np.ascontiguousarray(projections_b, dtype=np.float32)
    cc = np.ascontiguousarray(cluster_centers, dtype=np.float32)
    lab = np.asarray(labels).astype(np.float32)

    csc = np.ascontiguousarray((-2.0 * cc).T)
    csq = np.tile(np.sum(cc * cc, axis=1)[None, :], (128, 1)).astype(np.float32)
    i10r = np.tile(np.arange(C, dtype=np.float32)[None, :], (128, 1))
    i64m = np.tile((np.arange(K, dtype=np.float32) - K)[None, :], (128, 1))
    eye = np.eye(128, dtype=np.float32)

    _CACHE["aux"] = {"pa": pa, "pb": pb, "lab": lab.astype(np.int64)}
    in_maps = []
    for r in range(NC):
        s = slice(r * SH, (r + 1) * SH)
        labs = lab[s]
        ri = np.stack([((r + 1 + j) % NC) * 128 + np.arange(128) for j in range(4)],
                      axis=1).astype(np.int32)
        labc = labs.reshape(NCH, 128).T.astype(np.float32)   # [128, 8]
        in_maps.append({
            "pa": pa[s], "pb": pb[s], "ea": ea[s], "eb": eb[s],
            "encT": np.ascontiguousarray(np.concatenate([ea[s], eb[s]], 0).T),
            "centers_sc": csc, "csq": csq,
            "labels_f": np.ascontiguousarray(labc),
            "iota10r": i10r, "iota64m": i64m, "eye": eye,
            "rotidx": ri,
        })
    return in_maps


def _combine(results):
    outs = [np.asarray(res["out"], dtype=np.float64) for res in results]
    seg = np.asarray(results[0]["out2"], dtype=np.float64)

    # ---- l_main: scatter row-sum partials, assemble log-denominators ----
    aux = _CACHE["aux"]
    P = seg[0:C, 386:515]                    # class sums [10, 129]
    S_glob = np.zeros((128, 128))  # [block, row-in-block]
    for r in range(NC):
        s_oth = outs[r][:, 512:592]          # [128(p), 80(l)]
        g = (16 * r + np.arange(LBAND)) % 128
        np.add.at(S_glob, g, s_oth.T)
    l_main_sum = 0.0
    for r in range(NC):
        s = slice(r * SH, (r + 1) * SH)
        za = aux["pa"][s] / np.linalg.norm(aux["pa"][s], axis=1, keepdims=True)
        zb = aux["pb"][s] / np.linalg.norm(aux["pb"][s], axis=1, keepdims=True)
        zr = np.concatenate([za, zb], 0).astype(np.float64)       # [2048, 128]
        labs2 = np.concatenate([aux["lab"][s], aux["lab"][s]])
        a = np.sum(zr * P[labs2, 0:128], axis=1).reshape(NZ, 128).T
        n2 = P[labs2, 128].reshape(NZ, 128).T
        sm = outs[r][:, 592:608]             # [128, 16] col-side sums (my rows)
        sd = outs[r][:, 640:656]
        S = S_glob[16 * r:16 * r + 16].T + sm - np.exp((sd - 1.0) / T)
        m1 = (a - n2) / T / (n2 - 1.0)
        l_main_sum += np.sum(m1 - np.log(S))
    l_main = -(T / BT) * l_main_sum / (2 * B)

    # ---- l_p: prototype NTXent on host (64-dim, trivial) ----
    ca = np.maximum(seg[:, 256], 1.0)
    cb = np.maximum(seg[:, 385], 1.0)
    proto_a = seg[:, 0:128] / ca[:, None]
    proto_b = seg[:, 257:385] / cb[:, None]
    za = proto_a / np.linalg.norm(proto_a, axis=1, keepdims=True)
    zb = proto_b / np.linalg.norm(proto_b, axis=1, keepdims=True)
    z = np.concatenate([za, zb], 0)
    n = 2 * K
    sim = (z @ z.T) / T
    np.fill_diagonal(sim, -np.inf)
    pos = (np.arange(n) + K) % n
    mx = np.max(sim, axis=1, keepdims=True)
    logp = sim - mx - np.log(np.sum(np.exp(sim - mx), axis=1, keepdims=True))
    l_p = -np.mean(logp[np.arange(n), pos])

    # ---- l_n: meta CE from shipped -2*e.sup logits ----
    sup_v = seg[:, 128:256] / ca[:, None]
    ssq = np.sum(sup_v * sup_v, axis=1)
    l_n_sum = 0.0
    for r in range(NC):
        dn = outs[r][:, 0:512].reshape(128, NCH, K)
        xsq = outs[r][:, 656:664]
        lb = (outs[r][:, 664:680] + K)[:, NCH:].astype(np.int64)  # [128, 8]
        d2 = dn + xsq[:, :, None] + ssq[None, None, :]
        dd = np.sqrt(np.maximum(d2, 0.0))
        mxd = np.min(dd, axis=2, keepdims=True)
        ls = -(dd - mxd) - np.log(np.sum(np.exp(-(dd - mxd)), axis=2, keepdims=True))
        p_idx, c_idx = np.meshgrid(np.arange(128), np.arange(NCH), indexing="ij")
        l_n_sum += np.sum(ls[p_idx, c_idx, lb])
    l_n = -l_n_sum / B

    return np.float32(l_main + W_P * l_p + W_N * l_n)


def kernel(encodings_a, encodings_b, projections_a, projections_b,
           cluster_centers, labels):
    if "nc" not in _CACHE:
        _CACHE["nc"] = _build()
    nc = _CACHE["nc"]
    in_maps = _prep_inputs(encodings_a, encodings_b, projections_a,
                           projections_b, cluster_centers, labels)
    res = bass_utils.run_bass_kernel_spmd(nc, in_maps, core_ids=list(range(NC)))
    return _combine(res.results)
